# revision 1
# baseline (speedup 1.0000x reference)
"""Trainium2 Bass kernel for Direction-Aware Message Passing (GNN).

Full inputs in, full output out. Internally: data-parallel over images,
4 images per NeuronCore across 8 cores. See build_core_kernel for the
device-side dataflow.

Math (per image, n=80 nodes, e=1024 edges, D=1024):
  s = obj@Ws, o = obj@Wo (Wp folded into Ws on host)
  atten[e] = sum_d s[li[e],d]*o[lj[e],d]*phr[e,d]     (per-edge triple dot)
  A[li,lj] += atten (dup edges accumulate);  A = sigmoid(A); diag=0
  A[i,j] /= sum_m A[j,m]                                (reference quirk)
  c = relu(obj@Wc);  ctx = [A@c ; A^T@c]
  out = relu(obj + relu(LN(ctx@W1)*gamma+beta)@W2)      (gamma=1, beta=0)

Device techniques:
  - per-edge gathers s[li]/o[lj] via one-hot matmuls on the PE (k=80)
  - scatter-add into A via one-hot outer-product matmuls accumulating in PSUM
    (all 4 images' 80x80 accumulators packed into one PSUM bank)
  - triple product + reduction on DVE: tensor_tensor then
    scalar_tensor_tensor with accum_out (tensor_tensor_reduce crashes HW)
  - one-hot builds on GPSIMD (is_equal against iota constants)
  - normalization via row-sum reduce + reciprocal; N / N^T built with one
    per-partition scale plus PE transposes (no free-dim broadcasts)
  - ctx^T produced directly by per-image matmuls feeding the W1 matmul
"""

import os
import numpy as np
import ml_dtypes

import concourse.bacc as bacc
import concourse.bass as bass
import concourse.tile as tile
from concourse import masks, mybir
from concourse.bass_utils import run_bass_kernel_spmd

AF = mybir.ActivationFunctionType
ALU = mybir.AluOpType
DT = mybir.dt

B = 32          # images
NPI = 80        # nodes per image
EPI = 1024      # edges per image
D = 1024
NCORES = 8
IPC = B // NCORES          # images per core = 4
NODES = IPC * NPI          # 320
EDGES = IPC * EPI          # 4096
ET = 128                   # edges per tile
NTILES = EDGES // ET       # 32 edge tiles per core
TPI = NTILES // IPC        # 8 edge tiles per image
DH = D // 4                # 256
DC = D // 2                # 512
KD = D // 128              # 8 contraction k-tiles


def build_core_kernel(nc):
    f32, bf16 = DT.float32, DT.bfloat16
    t = {}
    t["obj"] = nc.dram_tensor("obj", [NODES, D], f32, kind="ExternalInput").ap()
    # phr packed per edge-tile, transposed: phr[t][q, c, e] = phr_orig[t*ET+e, c*128+q]
    t["phr"] = nc.dram_tensor("phr", [NTILES, ET, KD, ET], bf16,
                              kind="ExternalInput").ap()
    t["wS"] = nc.dram_tensor("wS", [D, D], bf16, kind="ExternalInput").ap()
    t["wO"] = nc.dram_tensor("wO", [D, D], bf16, kind="ExternalInput").ap()
    t["wC"] = nc.dram_tensor("wC", [D, DC], bf16, kind="ExternalInput").ap()
    t["w1"] = nc.dram_tensor("w1", [D, DH], bf16, kind="ExternalInput").ap()
    t["w2"] = nc.dram_tensor("w2", [DH, D], bf16, kind="ExternalInput").ap()
    # host-precomputed gather one-hot: onehotT(li) [80, E]
    t["ohg"] = nc.dram_tensor("ohg", [NPI, EDGES], bf16,
                              kind="ExternalInput").ap()
    # scatter one-hots per edge tile: ohs[0][e,t,i]=d(li), ohs[1]=d(lj)
    t["ohs"] = nc.dram_tensor("ohs", [2, ET, NTILES, NPI], bf16,
                              kind="ExternalInput").ap()
    t["dmask"] = nc.dram_tensor("dmask", [NPI, NPI], f32,
                                kind="ExternalInput").ap()
    t["out"] = nc.dram_tensor("out", [NODES, D], f32, kind="ExternalOutput").ap()
    reps = int(os.environ.get("KREPS", "1"))
    with tile.TileContext(nc) as tc:
        if reps > 1:
            with tc.For_i(0, reps, 1):
                _emit(nc, tc, t)
        else:
            _emit(nc, tc, t)
    nc.compile()


def _emit(nc, tc, td):
    from contextlib import ExitStack
    f32, bf16 = DT.float32, DT.bfloat16
    obj_d, phr_d, out_d = td["obj"], td["phr"], td["out"]
    with ExitStack() as ctx:
        const = ctx.enter_context(tc.tile_pool(name="const", bufs=1))
        wpool = ctx.enter_context(tc.tile_pool(name="w", bufs=1))
        objp = ctx.enter_context(tc.tile_pool(name="objp", bufs=1))
        feat = ctx.enter_context(tc.tile_pool(name="feat", bufs=1))
        asmall = ctx.enter_context(tc.tile_pool(name="asmall", bufs=3))
        phrp = ctx.enter_context(tc.tile_pool(name="phr", bufs=int(os.environ.get("KPHRBUFS", "6"))))
        ohp = ctx.enter_context(tc.tile_pool(name="oh", bufs=4))
        edges = ctx.enter_context(tc.tile_pool(name="edges", bufs=3))
        ctxp = ctx.enter_context(tc.tile_pool(name="ctxp", bufs=1))
        mlp = ctx.enter_context(tc.tile_pool(name="mlp", bufs=2))

        _kps = os.environ.get("KPS", "22")
        ps_misc = ctx.enter_context(
            tc.tile_pool(name="ps_misc", bufs=int(_kps[0]),
                         space=bass.MemorySpace.PSUM))
        ps_edge = ctx.enter_context(
            tc.tile_pool(name="ps_edge", bufs=int(_kps[1]),
                         space=bass.MemorySpace.PSUM))
        ps_A = ctx.enter_context(
            tc.tile_pool(name="ps_A", bufs=1, space=bass.MemorySpace.PSUM))

        if os.environ.get("KCOPY", "a") == "v":
            def CPc(out, in_):
                nc.vector.tensor_copy(out, in_)
        else:
            def CPc(out, in_):
                nc.scalar.copy(out, in_)

        # ---- small constants ----
        ident_b = const.tile([128, 128], bf16)
        masks.make_identity(nc, ident_b[:])
        epst = const.tile([128, 1], f32)
        nc.vector.memset(epst[:], 1e-5)
        dmask = const.tile([NPI, NPI], f32)         # 1 - eye(80), host-made
        nc.sync.dma_start(out=dmask[:], in_=td["dmask"][:, :])

        # ---- DMA emission order = issue priority: critical path first ----
        wS_s = wpool.tile([128, KD, D], bf16)
        wO_s = wpool.tile([128, KD, D], bf16)
        wC_s = wpool.tile([128, KD, DC], bf16)
        w1_s = wpool.tile([128, KD, DH], bf16)
        w2_s = wpool.tile([128, 2, D], bf16)
        for kc in range(KD):
            nc.sync.dma_start(out=wS_s[:, kc, :],
                              in_=td["wS"][kc * 128:(kc + 1) * 128, :])
        obj_img = []
        objb_img = []
        for b in range(IPC):
            of = objp.tile([NPI, D], f32, tag=f"objf{b}")
            nc.sync.dma_start(out=of[:, :], in_=obj_d[b * NPI:(b + 1) * NPI, :])
            ob = objp.tile([NPI, D], bf16, tag=f"objb{b}")
            CPc(ob[:, :], of[:, :])
            obj_img.append(of)
            objb_img.append(ob)
        ohli_s = const.tile([NPI, EDGES], bf16)     # gather one-hot (rhs)
        nc.sync.dma_start(out=ohli_s[:], in_=td["ohg"][:, :])
        for kc in range(KD):
            nc.sync.dma_start(out=wO_s[:, kc, :],
                              in_=td["wO"][kc * 128:(kc + 1) * 128, :])
        # prefetch first phr tiles ahead of the lower-priority weight loads
        phr_tiles = {}
        for gt in range(min(6, NTILES)):
            pt_ = phrp.tile([ET, KD, ET], bf16, tag="phr", name=f"phrpre{gt}")
            nc.sync.dma_start(out=pt_[:, :, :], in_=phr_d[gt, :, :, :])
            phr_tiles[gt] = pt_
        ohsl_s = const.tile([ET, NTILES, NPI], bf16)   # scatter lhsT one-hots
        ohsr_s = const.tile([ET, NTILES, NPI], bf16)   # scatter rhs one-hots
        nc.sync.dma_start(out=ohsl_s[:], in_=td["ohs"][0, :, :, :])
        nc.sync.dma_start(out=ohsr_s[:], in_=td["ohs"][1, :, :, :])
        for kc in range(KD):
            nc.sync.dma_start(out=wC_s[:, kc, :],
                              in_=td["wC"][kc * 128:(kc + 1) * 128, :])
        for kc in range(KD):
            nc.sync.dma_start(out=w1_s[:, kc, :],
                              in_=td["w1"][kc * 128:(kc + 1) * 128, :])
        for kh in range(2):
            nc.sync.dma_start(out=w2_s[:, kh, :],
                              in_=td["w2"][kh * 128:(kh + 1) * 128, :])

        # objT built per image from image-aligned obj tiles
        objT = objp.tile([128, KD, NODES], bf16)
        for b in range(IPC):
            for dc in range(KD):
                pt = ps_misc.tile([128, NPI], bf16, tag="ms")
                nc.tensor.transpose(pt[:, :], objb_img[b][:, dc * 128:(dc + 1) * 128],
                                    ident_b[:NPI, :NPI])
                CPc(objT[:, dc, b * NPI:(b + 1) * NPI], pt[:, :])

        o2T_all = feat.tile([128, KD, NODES], bf16, tag="o2T", name="o2T")
        for md in range(KD):
            ps = ps_misc.tile([128, NODES], f32, tag="ms", name="p1t")
            for kc in range(KD):
                nc.tensor.matmul(ps[:, :], wO_s[:, kc, md * 128:(md + 1) * 128],
                                 objT[:, kc, :],
                                 start=(kc == 0), stop=(kc == KD - 1))
            CPc(o2T_all[:, md, :], ps[:, :])

        ctxT_i = [ctxp.tile([128, KD, NPI], bf16, tag=f"ctxT{i}", name=f"ctxT{i}")
                  for i in range(IPC)]
        hT_i = [ctxp.tile([128, 2, NPI], bf16, tag=f"hT{i}", name=f"hT{i}")
                for i in range(IPC)]
        # all 4 images' A accumulators share one PSUM bank
        A4 = ps_A.tile([NPI, IPC, NPI], f32, tag="A", name="A4")

        feats = {}

        def emit_phase1(b):
            n0b = b * NPI
            with nc.named_scope(f"feat{b}"):
                s2 = feat.tile([NPI, D], bf16, tag=f"s2_{b}", name=f"s2_{b}")
                cfe = feat.tile([NPI, DC], bf16, tag=f"cf_{b}", name=f"cf_{b}")
                for (wt, dst, nn_tiles, act) in (
                        (wS_s, s2, 2, AF.Copy), (wC_s, cfe, 1, AF.Relu)):
                    for nn in range(nn_tiles):
                        ps = ps_misc.tile([NPI, 512], f32, tag="ms", name="p1")
                        for kc in range(KD):
                            nc.tensor.matmul(
                                ps[:, :], objT[:, kc, n0b:n0b + NPI],
                                wt[:, kc, nn * 512:(nn + 1) * 512],
                                start=(kc == 0), stop=(kc == KD - 1))
                        if act == AF.Copy:
                            CPc(dst[:, nn * 512:(nn + 1) * 512], ps[:, :])
                        else:
                            nc.scalar.activation(dst[:, nn * 512:(nn + 1) * 512],
                                                 ps[:, :], AF.Relu)
                feats[b] = (s2, cfe)

        def emit_mlp_out(b):
            n0b = b * NPI
            ctxT = ctxT_i[b]
            hT = hT_i[b]
            with nc.named_scope(f"mlp{b}"):
                pz = ps_misc.tile([NPI, DH], f32, tag="ms")
                for kc in range(KD):
                    nc.tensor.matmul(pz[:, :], ctxT[:, kc, :], w1_s[:, kc, :],
                                     start=(kc == 0), stop=(kc == KD - 1))
                stats = mlp.tile([NPI, 6], f32, tag="stats")
                nc.vector.bn_stats(stats[:, :], pz[:, :])
                mv = mlp.tile([NPI, 2], f32, tag="mv")
                nc.vector.bn_aggr(mv[:, :], stats[:, :])
                sd = mlp.tile([NPI, 1], f32, tag="sd")
                nc.scalar.activation(sd[:, :], mv[:, 1:2], AF.Sqrt,
                                     bias=epst[:NPI, :])
                rstd = mlp.tile([NPI, 1], f32, tag="rstd")
                nc.vector.reciprocal(rstd[:, :], sd[:, :])
                zt = mlp.tile([NPI, DH], f32, tag="zt")
                nc.vector.tensor_scalar(zt[:, :], pz[:, :], mv[:, 0:1],
                                        rstd[:, :], ALU.subtract, ALU.mult)
                ht = mlp.tile([NPI, DH], bf16, tag="ht")
                nc.scalar.activation(ht[:, :], zt[:, :], AF.Relu)
                for kh in range(2):
                    pt = ps_misc.tile([128, NPI], bf16, tag="ms")
                    nc.tensor.transpose(pt[:, :], ht[:, kh * 128:(kh + 1) * 128],
                                        ident_b[:NPI, :NPI])
                    CPc(hT[:, kh, :], pt[:, :])
            with nc.named_scope(f"out{b}"):
                fin = mlp.tile([NPI, D], f32, tag="fin")
                for nn in range(2):
                    po = ps_misc.tile([NPI, 512], f32, tag="ms")
                    for kh in range(2):
                        nc.tensor.matmul(po[:, :], hT[:, kh, :],
                                         w2_s[:, kh, nn * 512:(nn + 1) * 512],
                                         start=(kh == 0), stop=(kh == 1))
                    res = mlp.tile([NPI, 512], f32, tag="res")
                    nc.vector.tensor_tensor(res[:, :],
                                            obj_img[b][:, nn * 512:(nn + 1) * 512],
                                            po[:, :], ALU.add)
                    nc.scalar.activation(fin[:, nn * 512:(nn + 1) * 512],
                                         res[:, :], AF.Relu)
                nc.sync.dma_start(out=out_d[n0b:n0b + NPI, :], in_=fin[:, :])

        LOOK = int(os.environ.get("KLOOK", "1"))
        for b in range(min(LOOK, IPC)):
            emit_phase1(b)
        for b in range(IPC):
            n0 = b * NPI
            s2, cfe = feats[b]
            for t in range(TPI):                # 8 edge tiles per image
                gt = b * TPI + t
                e0 = gt * ET
                with nc.named_scope(f"edge{b}_{t}"):
                    if gt in phr_tiles:
                        phrt = phr_tiles.pop(gt)
                    else:
                        phrt = phrp.tile([ET, KD, ET], bf16, tag="phr")
                        nc.sync.dma_start(out=phrt[:, :, :],
                                          in_=phr_d[gt, :, :, :])
                    ohli = ohli_s[:, e0:e0 + ET]
                    sgT = ps_edge.tile([ET, KD, ET], f32, tag="g")
                    for c in range(KD):
                        nc.tensor.matmul(sgT[:, c, :],
                                         s2[:, c * 128:(c + 1) * 128], ohli,
                                         start=True, stop=True)
                    tmpT = edges.tile([ET, KD, ET], bf16, tag="tmp")
                    nc.vector.tensor_tensor(tmpT[:, :, :], sgT[:, :, :],
                                            phrt[:, :, :], ALU.mult)
                    M_ps = ps_edge.tile([ET, NPI], f32, tag="M", bufs=1)
                    for c in range(KD):
                        nc.tensor.matmul(M_ps[:, :], tmpT[:, c, :],
                                         o2T_all[:, c, n0:n0 + NPI],
                                         start=(c == 0), stop=(c == KD - 1))
                    R = ohp.tile([ET, NPI], bf16, tag="rhssc")
                    nc.vector.tensor_tensor(R[:], ohsr_s[:, gt, :], M_ps[:, :],
                                            ALU.mult)
                    nc.tensor.matmul(A4[:, b, :], ohsl_s[:, gt, :], R[:],
                                     start=(t == 0), stop=(t == TPI - 1),
                                     skip_group_check=True)

            if b + LOOK < IPC:
                emit_phase1(b + LOOK)
            with nc.named_scope(f"attn{b}"):
                A_sig = asmall.tile([NPI, NPI], f32, tag="asig")
                nc.scalar.activation(A_sig[:], A4[:, b, :], AF.Sigmoid)
                A_s = asmall.tile([NPI, NPI], bf16, tag="as")
                nc.vector.tensor_tensor(A_s[:], A_sig[:], dmask[:], ALU.mult)
                r_col = asmall.tile([NPI, 1], f32, tag="r")
                nc.vector.tensor_reduce(r_col[:], A_s[:], mybir.AxisListType.X,
                                        ALU.add)
                rinv = asmall.tile([NPI, 1], f32, tag="rinv")
                nc.vector.reciprocal(rinv[:], r_col[:])
                pT = ps_misc.tile([NPI, NPI], bf16, tag="ms")
                nc.tensor.transpose(pT[:, :], A_s[:], ident_b[:NPI, :NPI])
                NT_t = asmall.tile([NPI, NPI], bf16, tag="NT")
                nc.vector.tensor_scalar(NT_t[:], pT[:, :], rinv[:], None, ALU.mult)
                pN = ps_misc.tile([NPI, NPI], bf16, tag="ms")
                nc.tensor.transpose(pN[:, :], NT_t[:], ident_b[:NPI, :NPI])
                N_t = asmall.tile([NPI, NPI], bf16, tag="N")
                CPc(N_t[:], pN[:, :])
                ctxT = ctxT_i[b]
                for md in range(4):
                    pc = ps_misc.tile([128, NPI], f32, tag="ms")
                    nc.tensor.matmul(pc[:, :], cfe[:, md * 128:(md + 1) * 128],
                                     NT_t[:], start=True, stop=True)
                    CPc(ctxT[:, md, :], pc[:, :])
                    pc2 = ps_misc.tile([128, NPI], f32, tag="ms")
                    nc.tensor.matmul(pc2[:, :], cfe[:, md * 128:(md + 1) * 128],
                                     N_t[:], start=True, stop=True)
                    CPc(ctxT[:, 4 + md, :], pc2[:, :])
            emit_mlp_out(b)


_COMPILED = {}


def _get_compiled():
    if "nc" not in _COMPILED:
        nc = bacc.Bacc("TRN2", target_bir_lowering=False, debug=False)
        build_core_kernel(nc)
        _COMPILED["nc"] = nc
    return _COMPILED["nc"]


def _prep_shards(obj_feats, phr_feats, im_inds, rel_inds,
                 Ws, bs, Wo, bo, Wp, bp, Wc, bc, W1, b1, gamma, beta, W2, b2):
    obj_feats = np.ascontiguousarray(obj_feats, dtype=np.float32)
    phr_feats = np.ascontiguousarray(phr_feats, dtype=np.float32)
    rel = np.asarray(rel_inds).astype(np.int64)
    for name, v, ref in (("bs", bs, 0), ("bo", bo, 0), ("bp", bp, 0),
                         ("bc", bc, 0), ("b1", b1, 0), ("b2", b2, 0),
                         ("beta", beta, 0), ("gamma", gamma, 1)):
        if not np.allclose(np.asarray(v), ref, atol=1e-30):
            raise NotImplementedError(f"nonzero {name} not supported")

    b = rel[:, 0]
    if not np.array_equal(b, np.repeat(np.arange(B), EPI)):
        order = np.argsort(b, kind="stable")
        counts = np.bincount(b[order].astype(np.int64), minlength=B)
        if not np.all(counts == EPI):
            raise NotImplementedError("uneven edges per image")
        rel = rel[order]
        phr_feats = np.ascontiguousarray(phr_feats[order])
        b = rel[:, 0]
    li = (rel[:, 1] - b * NPI).astype(np.int64)
    lj = (rel[:, 2] - b * NPI).astype(np.int64)
    assert li.min() >= 0 and li.max() < NPI and lj.min() >= 0 and lj.max() < NPI

    bf = ml_dtypes.bfloat16
    Ws_eff = (np.asarray(Ws, np.float32) *
              np.asarray(Wp, np.float32)[:, 0][None, :]).astype(bf)
    Wo_b = np.asarray(Wo, np.float32).astype(bf)
    Wc_b = np.asarray(Wc, np.float32).astype(bf)
    W1_b = np.asarray(W1, np.float32).astype(bf)
    W2_b = np.asarray(W2, np.float32).astype(bf)

    dmask = (1.0 - np.eye(NPI)).astype(np.float32)
    ar = np.arange(NPI)
    in_maps = []
    for core in range(NCORES):
        esl = slice(core * EDGES, (core + 1) * EDGES)
        li_c, lj_c = li[esl], lj[esl]
        ohg = (li_c[None, :] == ar[:, None]).astype(bf)
        # [2, ET, NTILES, NPI]: [k][e, t, i] = onehot over i, edge = t*ET+e
        ohs = np.stack([
            (li_c.reshape(NTILES, ET)[None, :, :] ==
             ar[:, None, None]).transpose(2, 1, 0),
            (lj_c.reshape(NTILES, ET)[None, :, :] ==
             ar[:, None, None]).transpose(2, 1, 0)]).astype(bf)
        # pack [NTILES, ET, KD, ET]: phr_p[t, q, c, e] = phr[t*ET+e, c*128+q]
        phr_host = (phr_feats[esl].reshape(NTILES, ET, KD, 128)
                    .transpose(0, 3, 2, 1).astype(bf))
        in_maps.append(dict(
            obj=np.ascontiguousarray(obj_feats[core * NODES:(core + 1) * NODES]),
            phr=np.ascontiguousarray(phr_host),
            wS=Ws_eff, wO=Wo_b, wC=Wc_b, w1=W1_b, w2=W2_b,
            ohg=np.ascontiguousarray(ohg),
            ohs=np.ascontiguousarray(ohs),
            dmask=dmask,
        ))
    return in_maps


def kernel(**inputs):
    nc = _get_compiled()
    in_maps = _prep_shards(**inputs)
    res = run_bass_kernel_spmd(nc, in_maps, list(range(NCORES)))
    out = np.concatenate([res.results[c]["out"] for c in range(NCORES)], axis=0)
    kernel.last_results = res
    return out

